# revision 18
# baseline (speedup 1.0000x reference)
"""Multi-head attention (dense_transformer) on 8 TRN2 NeuronCores.

Sharding: 2-way data parallel over batch x 4-way tensor parallel over heads.
Core c handles batch b=c//4 and heads {4g..4g+3} where g=c%4 (4 heads, 256
channels per core; channels of head h are qw columns {hd*16+h}).

Per core:
  phase 1: Q^T/K^T/V^T projections ([ch, s] layout, fp32 PE matmuls)
  phase 2: causal flash-style attention per head: scores [q,k] in PSUM (fp32),
           rowmax -> exp(bias=-max) on ACT (accum row sums) -> normalize by
           1/sum -> PE-transpose normalized attn -> O^T = V^T@attnT
           (two heads col-packed into one PSUM tile via tile_position)
  phase 3: AllGather O^T across the 4 cores of the batch -> out-proj column
           slice (out^T = ow_perm^T @ merged^T, float32r) -> DMA out.

dtype notes (measured on HW): float32r matmul = ~11-bit mantissa; declaring a
DRAM tensor float32r poisons even "fp32" matmuls on it (bitcast doesn't help),
so precision-critical tensors (x, qw, kw, vw) are declared float32 and the
Q/K/scores path runs true fp32. The out-projection path (wo, O^T, allgather)
stays float32r: its error contribution is linear and tiny.
"""
import sys

sys.path.insert(0, "/opt/trn_rl_repo")

import numpy as np

import concourse.bass as bass
import concourse.mybir as mybir
import concourse.tile as tile
from concourse import bacc
from concourse.bass_utils import run_bass_kernel_spmd
from concourse.masks import make_causal_mask, make_identity

# ---- problem constants (hardcoded per harness contract) ----
B, S, D, HEADS = 2, 2048, 1024, 16
N_CORES = 8
GROUPS = 4                 # head-groups == cores per batch
HPC = HEADS // GROUPS      # heads per core (4)
HD = D // HEADS            # 64
CPC = HPC * HD             # channels per core (256)
P = 128
NCC = CPC // P             # col chunks per core (2)
DCH = D // P               # contraction chunks (8)

f32 = mybir.dt.float32
f32r = mybir.dt.float32r
bf16 = mybir.dt.bfloat16

AX = mybir.AxisListType
EXP = mybir.ActivationFunctionType.Exp

DEFAULT_CFG = dict(
    s=S,
    attn_dt="bf16",    # "bf16" | "f32" for attn weights / V / AV matmul
    gw=512,            # AV group width (q columns per O^T psum tile)
)


def build_nc(s=S, attn_dt="bf16", gw=256, dbg=False):
    assert s % 512 == 0
    NQI = s // P           # q chunks of 128
    NQB = s // 512         # 512-wide q blocks (projection / scores k blocks)
    QPG = gw // P          # q chunks per AV group
    NGRP = s // gw         # AV groups

    att_dt = {"bf16": bf16, "f32": f32}[attn_dt]
    del attn_dt

    nc = bacc.Bacc("TRN2", target_bir_lowering=False, debug=False,
                   num_devices=N_CORES)
    xT = nc.dram_tensor("xT", [D, s], f32, kind="ExternalInput").ap()
    wq = nc.dram_tensor("wq", [D, CPC], f32, kind="ExternalInput").ap()
    wk = nc.dram_tensor("wk", [D, CPC], f32, kind="ExternalInput").ap()
    wv = nc.dram_tensor("wv", [D, CPC], f32, kind="ExternalInput").ap()
    wo = nc.dram_tensor("wo", [D, CPC], f32r, kind="ExternalInput").ap()
    outT = nc.dram_tensor("outT", [NCC, P, s], f32, kind="ExternalOutput").ap()
    dbg_t = {}
    if dbg:
        for nm, shp in (("QT", [P, NCC, s]), ("KT", [P, NCC, s]),
                        ("Vsb", [P, s // P, CPC]), ("OT", [P, NCC, s]),
                        ("at0", [P, s]), ("sc0", [P, 4, 512]),
                        ("atT0", [P, s // P, gw])):
            dbg_t[nm] = nc.dram_tensor("dbg_" + nm, shp, f32,
                                       kind="ExternalOutput").ap()

    with tile.TileContext(nc) as tc:
        with (
            tc.tile_pool(name="cpool", bufs=1) as cpool,
            tc.tile_pool(name="wpool", bufs=1) as wpool,
            tc.tile_pool(name="big", bufs=1) as big,
            tc.tile_pool(name="xs", bufs=6) as xs,
            tc.tile_pool(name="apool", bufs=3) as apool,
            tc.tile_pool(name="atp", bufs=1) as atp,
            tc.tile_pool(name="stat", bufs=8) as stat,
            tc.tile_pool(name="scb", bufs=6) as scbp,
            tc.tile_pool(name="ms", bufs=6) as ms,
            tc.tile_pool(name="op", bufs=3) as op,
            tc.tile_pool(name="dram", bufs=1, space="DRAM") as dpool,
        ):
            ag_in = [dpool.tile([P, s], f32r, tag=f"agin{hp}", name=f"agin{hp}")
                     for hp in range(NCC)]
            ag_out = [dpool.tile([GROUPS, P, s], f32r, tag=f"agout{hp}",
                                 name=f"agout{hp}")
                      for hp in range(NCC)]

            Wmask = cpool.tile([P, P], f32, tag="Wmask")
            make_causal_mask(nc, Wmask[:], mask_val=-1e10)
            ident = cpool.tile([P, P], att_dt, tag="ident")
            make_identity(nc, ident[:])

            wo_sb = wpool.tile([P, DCH, CPC], f32r, tag="wo")
            nc.sync.dma_start(wo_sb[:], wo.rearrange("(o p) c -> p o c", p=P))
            wsplit = {}
            for nm, wdr in (("q", wq), ("k", wk), ("v", wv)):
                wh = wpool.tile([P, DCH, CPC], bf16, tag=f"w{nm}h", name=f"w{nm}h")
                wl = wpool.tile([P, DCH, CPC], bf16, tag=f"w{nm}l", name=f"w{nm}l")
                wsplit[nm] = [wh, wl]
            with tc.tile_pool(name="wload", bufs=1) as wload:
                for nm, wdr in (("q", wq), ("k", wk), ("v", wv)):
                    wf = wload.tile([P, DCH, CPC], f32, tag="wf", name="wf")
                    nc.sync.dma_start(wf[:], wdr.rearrange("(o p) c -> p o c", p=P))
                    wh, wl = wsplit[nm]
                    nc.vector.tensor_copy(wh[:], wf[:])
                    nc.vector.tensor_tensor(wl[:], wf[:], wh[:],
                                            mybir.AluOpType.subtract)

            QTh = big.tile([P, NCC, s], bf16, tag="QTh")
            QTl = big.tile([P, NCC, s], bf16, tag="QTl")
            KTh = big.tile([P, NCC, s], bf16, tag="KTh")
            KTl = big.tile([P, NCC, s], bf16, tag="KTl")
            VTb = big.tile([P, NCC, s], att_dt, tag="VTb")
            Vsb = big.tile([P, s // P, CPC], att_dt, tag="Vsb")
            OT = big.tile([P, NCC, s], f32r, tag="OT")

            # ---------------- phase 1: projections (fp32) ----------------
            with tc.tile_pool(name="psp", bufs=2, space="PSUM") as psp:
                for qb in range(NQB):
                    accs = {}
                    for nm in ("q", "k", "v"):
                        for cc in range(NCC):
                            accs[nm, cc] = psp.tile([P, 512], f32,
                                                    tag=f"pp{nm}", name=f"pp{nm}{cc}")
                    for di in range(DCH):
                        xt = xs.tile([P, 512], f32, tag="xt", name="xt")
                        nc.sync.dma_start(
                            xt[:], xT[di * P:(di + 1) * P, qb * 512:(qb + 1) * 512])
                        xth = xs.tile([P, 512], bf16, tag="xth", name="xth")
                        xtl = xs.tile([P, 512], bf16, tag="xtl", name="xtl")
                        nc.vector.tensor_copy(xth[:], xt[:])
                        nc.vector.tensor_tensor(xtl[:], xt[:], xth[:],
                                                mybir.AluOpType.subtract)
                        for nm in ("q", "k", "v"):
                            wh, wl = wsplit[nm]
                            for cc in range(NCC):
                                csl = slice(cc * P, (cc + 1) * P)
                                terms = [(wh, xth), (wh, xtl), (wl, xth)]
                                for ti, (wt, xtt) in enumerate(terms):
                                    nc.tensor.matmul(
                                        accs[nm, cc][:], wt[:, di, csl], xtt[:],
                                        start=(di == 0 and ti == 0),
                                        stop=(di == DCH - 1 and ti == len(terms) - 1))
                    sl = slice(qb * 512, (qb + 1) * 512)
                    for cc in range(NCC):
                        for hi_t, lo_t, ps in ((QTh, QTl, accs["q", cc]),
                                               (KTh, KTl, accs["k", cc])):
                            nc.any.tensor_copy(hi_t[:, cc, sl], ps[:])
                            nc.vector.tensor_tensor(lo_t[:, cc, sl], ps[:],
                                                    hi_t[:, cc, sl],
                                                    mybir.AluOpType.subtract)
                        nc.any.tensor_copy(VTb[:, cc, sl], accs["v", cc][:])

            # ---------------- phase 2: attention ----------------
            with (
                tc.tile_pool(name="pssc", bufs=4, space="PSUM") as pssc,
                tc.tile_pool(name="pspt", bufs=3, space="PSUM") as pspt,
                tc.tile_pool(name="psot", bufs=1, space="PSUM") as psot,
            ):
                # V^T -> V (PE transposes)
                for cc in range(NCC):
                    for ki in range(s // P):
                        pt = pspt.tile([P, P], att_dt, tag="pt", name="ptv")
                        nc.tensor.transpose(pt[:], VTb[:, cc, ki * P:(ki + 1) * P],
                                            ident[:])
                        nc.any.tensor_copy(Vsb[:, ki, cc * P:(cc + 1) * P], pt[:])

                for hp in range(NCC):          # head pair == col chunk
                    hp_grps = range(NGRP)
                    for grp in hp_grps:
                        atT = {}
                        for h2 in range(2):
                            atT[h2] = atp.tile([P, s // P, gw], att_dt,
                                               tag=f"atT{h2}", name=f"atT{h2}")
                            # zero invalid (future-k) diag regions
                            for dk in range(1, QPG):
                                ki = grp * QPG + dk
                                nc.any.memset(atT[h2][:, ki, 0:dk * P], 0.0)
                        for r in range(QPG):
                            qi = grp * QPG + r
                            nkb = qi // 4 + 1
                            wlast = (qi % 4 + 1) * P
                            for h2 in range(2):
                                hsl = slice(h2 * 64, (h2 + 1) * 64)
                                qsl_h = QTh[hsl, hp, qi * P:(qi + 1) * P]
                                qsl_l = QTl[hsl, hp, qi * P:(qi + 1) * P]
                                sc_tiles = []
                                for j in range(nkb):
                                    wj = 512 if j < nkb - 1 else wlast
                                    st = pssc.tile([P, 512], f32, tag="sc", name="sc")
                                    sc_tiles.append((st, wj))
                                # term-outer so the stationary Q operand is
                                # reused across k-blocks (fewer LDWEIGHTS)
                                for ti, (qq, kside) in enumerate(
                                        ((qsl_h, KTh), (qsl_h, KTl), (qsl_l, KTh))):
                                    for j, (st, wj) in enumerate(sc_tiles):
                                        kk = kside[hsl, hp,
                                                   j * 512:j * 512 + wj]
                                        nc.tensor.matmul(st[:, :wj], qq, kk,
                                                         start=(ti == 0),
                                                         stop=(ti == 2))
                                # drain scores to SBUF so PSUM banks recycle
                                # fast and iterations pipeline on the PE
                                sc_sb = []
                                for j, (st, wj) in enumerate(sc_tiles):
                                    sb_t = scbp.tile([P, 512], f32, tag="scb",
                                                     name="scb")
                                    nc.any.tensor_copy(sb_t[:, :wj], st[:, :wj])
                                    sc_sb.append((sb_t, wj))
                                sc_tiles = sc_sb
                                # additive causal mask on the diagonal subtile
                                last, wl = sc_tiles[-1]
                                nc.vector.tensor_add(last[:, wl - P:wl],
                                                     last[:, wl - P:wl], Wmask[:])
                                # row stats
                                mc = stat.tile([P, 4], f32, tag="mc", name="mc")
                                for j, (st, wj) in enumerate(sc_tiles):
                                    nc.vector.reduce_max(mc[:, j:j + 1], st[:, :wj],
                                                         axis=AX.X)
                                negm = stat.tile([P, 1], f32, tag="negm", name="negm")
                                if nkb > 1:
                                    m = stat.tile([P, 1], f32, tag="m", name="m")
                                    nc.vector.reduce_max(m[:], mc[:, :nkb], axis=AX.X)
                                    nc.vector.tensor_scalar_mul(negm[:], m[:], -1.0)
                                else:
                                    nc.vector.tensor_scalar_mul(negm[:], mc[:, 0:1],
                                                                -1.0)
                                dump_this = (dbg and hp == 0 and h2 == 0
                                             and qi == min(8, NQI - 1))
                                if dump_this:
                                    for j, (st, wj) in enumerate(sc_tiles):
                                        dsc = stat.tile([P, 512], f32, tag="dsc",
                                                        name="dsc")
                                        nc.vector.tensor_copy(dsc[:, :wj], st[:, :wj])
                                        nc.sync.dma_start(dbg_t["sc0"][:, j, :wj],
                                                          dsc[:, :wj])
                                at = apool.tile([P, s], att_dt, tag="at", name="at")
                                sums = stat.tile([P, 4], f32, tag="sums", name="sums")
                                for j, (st, wj) in enumerate(sc_tiles):
                                    nc.scalar.activation(
                                        at[:, j * 512:j * 512 + wj], st[:, :wj], EXP,
                                        bias=negm[:], accum_out=sums[:, j:j + 1])
                                Ssum = stat.tile([P, 1], f32, tag="Ssum", name="Ssum")
                                nc.vector.reduce_sum(Ssum[:], sums[:, :nkb], axis=AX.X)
                                rec = stat.tile([P, 1], f32, tag="rec", name="rec")
                                nc.vector.reciprocal(rec[:], Ssum[:])
                                ktot = (qi + 1) * P
                                nc.any.tensor_scalar_mul(at[:, :ktot], at[:, :ktot],
                                                         rec[:])
                                if dump_this:
                                    for ki2 in range(qi + 1):
                                        dat = op.tile([P, P], f32, tag="dat",
                                                      name="dat")
                                        nc.any.tensor_copy(
                                            dat[:], at[:, ki2 * P:(ki2 + 1) * P])
                                        nc.sync.dma_start(
                                            dbg_t["at0"][:, ki2 * P:(ki2 + 1) * P],
                                            dat[:])
                                for ki in range(qi + 1):
                                    pt = pspt.tile([P, P], att_dt, tag="pt",
                                                   name="pta")
                                    nc.tensor.transpose(
                                        pt[:], at[:, ki * P:(ki + 1) * P], ident[:])
                                    nc.any.tensor_copy(
                                        atT[h2][:, ki, r * P:(r + 1) * P], pt[:])
                        # AV for this group (two heads col-packed)
                        nch = grp * QPG + QPG
                        otp = psot.tile([P, gw], f32, tag="ot", name="otp")
                        for h2 in range(2):
                            vcols = slice(hp * P + h2 * 64, hp * P + (h2 + 1) * 64)
                            for ki in range(nch):
                                nc.tensor.matmul(
                                    otp[h2 * 64:(h2 + 1) * 64, :],
                                    Vsb[:, ki, vcols], atT[h2][:, ki, :],
                                    start=(ki == 0), stop=(ki == nch - 1),
                                    tile_position=(0, h2 * 64))
                        nc.any.tensor_copy(OT[:, hp, grp * gw:(grp + 1) * gw], otp[:])
                        if dbg and hp == 0 and grp == min(8, NQI - 1) // QPG:
                            for ki in range(s // P):
                                cv = op.tile([P, gw], f32, tag="cv3", name="cv3")
                                nc.any.tensor_copy(cv[:], atT[0][:, ki])
                                nc.sync.dma_start(dbg_t["atT0"][:, ki], cv[:])
                    # this head pair's O^T is complete: gather it now so the
                    # collective overlaps the next head pair's compute
                    nc.sync.dma_start(ag_in[hp][:], OT[:, hp, :])
                    nc.gpsimd.collective_compute(
                        "AllGather", mybir.AluOpType.bypass,
                        replica_groups=[[0, 1, 2, 3], [4, 5, 6, 7]],
                        ins=[ag_in[hp][:]], outs=[ag_out[hp][:]],
                    )

            if dbg:
                for nm, t in (("QT", QTh), ("KT", KTh), ("OT", OT)):
                    tv = t.bitcast(f32) if t.dtype != f32 else t
                    for cc in range(NCC):
                        for sb2 in range(s // 512):
                            cv = op.tile([P, 512], f32, tag="cv", name="cv")
                            nc.any.tensor_copy(cv[:],
                                               tv[:, cc, sb2 * 512:(sb2 + 1) * 512])
                            nc.sync.dma_start(
                                dbg_t[nm][:, cc, sb2 * 512:(sb2 + 1) * 512], cv[:])
                for ki in range(s // P):
                    cv = op.tile([P, CPC], f32, tag="cv2", name="cv2")
                    nc.any.tensor_copy(cv[:], Vsb[:, ki])
                    nc.sync.dma_start(dbg_t["Vsb"][:, ki], cv[:])

            # ------------- phase 3: AllGather + out-proj (f32r) -------------
            with tc.tile_pool(name="pso", bufs=2, space="PSUM") as pso:
                for sb_ in range(s // 512):
                    ssl = slice(sb_ * 512, (sb_ + 1) * 512)
                    accs = [pso.tile([P, 512], f32, tag="po", name=f"po{occ}")
                            for occ in range(NCC)]
                    for mch in range(DCH):
                        g_, cc_ = mch // NCC, mch % NCC
                        mt = ms.tile([P, 512], f32r, tag="mt", name="mt")
                        nc.sync.dma_start(mt[:], ag_out[cc_][g_, :, ssl])
                        for occ in range(NCC):
                            nc.tensor.matmul(
                                accs[occ][:], wo_sb[:, mch, occ * P:(occ + 1) * P],
                                mt[:], start=(mch == 0), stop=(mch == DCH - 1))
                    for occ in range(NCC):
                        oo = op.tile([P, 512], f32, tag="oo", name="oo")
                        nc.any.tensor_copy(oo[:], accs[occ][:])
                        nc.sync.dma_start(outT[occ, :, ssl], oo[:])

    nc.compile()
    return nc


_NC_CACHE = {}


def get_nc(**cfg):
    key = tuple(sorted(cfg.items()))
    if key not in _NC_CACHE:
        _NC_CACHE[key] = build_nc(**cfg)
    return _NC_CACHE[key]


def _col_index(g):
    p = np.arange(CPC)
    return (p % HD) * HEADS + (HPC * g + p // HD)


def _ow_row_index():
    r = np.arange(D)
    m, p128 = r // P, r % P
    g_, cc = m // NCC, m % NCC
    p256 = cc * P + p128
    lh, hd = p256 // HD, p256 % HD
    return hd * HEADS + (HPC * g_ + lh)


def make_in_maps(x, qw, kw, vw, ow, s=S):
    scale = 1.0 / np.sqrt(np.float32(D))
    qws = (qw * scale).astype(np.float32)
    ow_perm = np.ascontiguousarray(ow[_ow_row_index()])
    in_maps = []
    xTs = [np.ascontiguousarray(x[b, :s].T) for b in range(B)]
    for c in range(N_CORES):
        b, g = c // GROUPS, c % GROUPS
        cols = _col_index(g)
        in_maps.append({
            "xT": xTs[b],
            "wq": np.ascontiguousarray(qws[:, cols]),
            "wk": np.ascontiguousarray(kw[:, cols]),
            "wv": np.ascontiguousarray(vw[:, cols]),
            "wo": np.ascontiguousarray(ow_perm[:, g * CPC:(g + 1) * CPC]),
        })
    return in_maps


def assemble_output(results, s=S):
    out = np.empty((B, s, D), dtype=np.float32)
    for c in range(N_CORES):
        b, g = c // GROUPS, c % GROUPS
        oT = results[c]["outT"]  # [NCC, P, s]
        for occ in range(NCC):
            out[b, :, g * CPC + occ * P:(g * CPC + (occ + 1) * P)] = oT[occ].T
    return out


def run_on_hw(x, qw, kw, vw, ow, trace=False, **cfg_over):
    cfg = dict(DEFAULT_CFG)
    cfg.update(cfg_over)
    s = cfg["s"]
    nc = get_nc(**cfg)
    in_maps = make_in_maps(x, qw, kw, vw, ow, s=s)
    res = run_bass_kernel_spmd(nc, in_maps, core_ids=list(range(N_CORES)),
                               trace=trace)
    return assemble_output(res.results, s=s), res


def kernel(x, qw, kw, vw, ow):
    out, _ = run_on_hw(np.asarray(x, dtype=np.float32),
                       np.asarray(qw, dtype=np.float32),
                       np.asarray(kw, dtype=np.float32),
                       np.asarray(vw, dtype=np.float32),
                       np.asarray(ow, dtype=np.float32))
    return out
